# revision 24
# baseline (speedup 1.0000x reference)
"""MoE (top-2 of 8 experts, SwiGLU) kernel for 8 Trainium2 NeuronCores.

Strategy (expert-parallel, matching the sharding hint):
  - Router runs on host (tiny: [4096,512]@[512,8]); tokens are dispatched to
    the core owning their routed expert ("all-to-all dispatch" done at shard
    time since this contract takes full inputs on host anyway).
  - Core e holds expert e's gate/up/down weights and computes the SwiGLU FFN
    over the tokens routed to it (zero-padded to a fixed capacity C).
  - The weighted combine (top-2 probabilities) + scatter-add back to token
    order happens at unshard time, with the aux losses (scalars) from the
    host router pass.

Device kernel layout: activations kept transposed [feature, token] so all
three matmuls chain without transposes:
  h = silu(Wg^T x) * (Wu^T x)   [F=1024, tokens]
  y = Wd^T h                    [D=512, tokens]
Matmuls run as float32r (full-rate fp32 path, N>=256 per chunk).
"""

import numpy as np

B, S, D, F, E, K = 4, 1024, 512, 1024, 8, 2
T = B * S
LB_W, Z_W = 0.01, 0.001

C = 1152                      # per-expert token capacity (max observed ~1092)
CHUNKS = [(0, 512), (512, 384), (896, 256)]
DMA_PLAN = [("x", 0, "sp"), ("wg", 0, "sp"), ("wu", 0, "sp"), ("wg", 1, "sp"),
            ("wu", 1, "sp"), ("x", 12, "sp"), ("wd", 0, "sp"), ("wd", 1, "sp")]
PSG_BUFS, PSU_BUFS, PSY_BUFS = 2, 2, 3
SIL_BUFS, H_BUFS, Y_BUFS = 3, 2, 3
ORDER = "chunk_major"   # or "f_major" / "fused"
XSPLIT = False          # interleave per-do x/wg startup DMAs
N_CORES = 8

TRACE = False                 # set by test harness to get HW exec time
LAST_EXEC_NS = None

_NC = None


def _build(loop_n=1):
    import contextlib
    import concourse.bacc as bacc
    import concourse.tile as tile
    import concourse.mybir as mybir

    f32 = mybir.dt.float32
    f32r = mybir.dt.float32r
    SILU = mybir.ActivationFunctionType.Silu

    nc = bacc.Bacc("TRN2", target_bir_lowering=False, debug=False,
                   num_devices=N_CORES)
    # Inputs declared float32r: the DMA then feeds the fp32r matmuls directly
    # (numpy side still float32 — same bits, HW rounds internally).
    xT = nc.dram_tensor("xT", [D, C], f32r, kind="ExternalInput").ap()
    wg = nc.dram_tensor("wg", [D, F], f32r, kind="ExternalInput").ap()
    wu = nc.dram_tensor("wu", [D, F], f32r, kind="ExternalInput").ap()
    wd = nc.dram_tensor("wd", [F, D], f32r, kind="ExternalInput").ap()
    yT = nc.dram_tensor("yT", [D, C], f32, kind="ExternalOutput").ap()

    nd = D // 128   # 4 d-tiles
    nf = F // 128   # 8 f-tiles

    with tile.TileContext(nc) as tc:
        with (
            tc.tile_pool(name="wp", bufs=1) as wpool,
            tc.tile_pool(name="xp", bufs=1) as xpool,
            tc.tile_pool(name="hp", bufs=H_BUFS) as hpool,
            tc.tile_pool(name="yp", bufs=Y_BUFS) as ypool,
            tc.tile_pool(name="sp", bufs=SIL_BUFS) as spool,
            tc.tile_pool(name="psg", bufs=PSG_BUFS, space="PSUM") as psg,
            tc.tile_pool(name="psu", bufs=PSU_BUFS, space="PSUM") as psu,
            tc.tile_pool(name="psy", bufs=PSY_BUFS, space="PSUM") as psy,
        ):
          with tc.For_i(0, loop_n, 1) if loop_n > 1 else contextlib.nullcontext():
            xs = xpool.tile([128, nd, C], f32r)
            wgs = wpool.tile([128, nd, F], f32r, tag="wg")
            wus = wpool.tile([128, nd, F], f32r, tag="wu")
            wds = wpool.tile([128, nf, D], f32r, tag="wd")
            # Bulk DMAs through rearranged DRAM views ([(n p) c -> p n c]) —
            # DMA issue occupies the SP sequencer ~0.6us per dma_start, so few
            # big transfers beat many small ones. Order: x chunk 0 and the
            # per-do wg/wu slabs first so the first chunk's matmuls can start
            # after ~3MB instead of the full 11MB.
            xr = xT.rearrange("(n p) c -> p n c", p=128)
            wgr = wg.rearrange("(n p) f -> p n f", p=128)
            wur = wu.rearrange("(n p) f -> p n f", p=128)
            wdr = wd.rearrange("(n p) d -> p n d", p=128)
            yr = yT.rearrange("(n p) c -> p n c", p=128)
            if ORDER == "fused" or XSPLIT:
                # Interleave per-do x / wg loads so the first matmul can
                # start after just the do=0 pieces (~3.3us) instead of the
                # whole x-chunk0 + wg slabs.
                for do in range(nd):
                    nc.sync.dma_start(xs[:, do, 0:512], xr[:, do, 0:512])
                    nc.sync.dma_start(wgs[:, do, :], wgr[:, do, :])
            else:
                nc.sync.dma_start(xs[:, :, 0:512], xr[:, :, 0:512])
                for do in range(nd):
                    nc.sync.dma_start(wgs[:, do, :], wgr[:, do, :])
            for do in range(nd):
                nc.sync.dma_start(wus[:, do, :], wur[:, do, :])

            nc.sync.dma_start(xs[:, :, 512:C], xr[:, :, 512:C])
            nc.sync.dma_start(wds[:], wdr[:, :, :])

            if ORDER == "fused":
                # Down-projection matmuls ride inside the f-loop: 4 PSUM
                # accumulation groups (one per d-tile) stay open across the
                # whole chunk, so down(f) issues right after h(f) instead of
                # the whole down phase waiting on the last f-tile's silu*mul.
                for (c0, cw) in CHUNKS:
                    y_l = [psy.tile([128, 512], f32, tag="yps",
                                    name=f"yps{c0}_{do}") for do in range(nd)]
                    for f in range(nf):
                        g_ps = psg.tile([128, 512], f32)
                        u_ps = psu.tile([128, 512], f32)
                        fs = slice(f * 128, (f + 1) * 128)
                        for do in range(nd):
                            nc.tensor.matmul(
                                g_ps[:, :cw],
                                wgs[:, do, fs],
                                xs[:, do, c0:c0 + cw],
                                start=(do == 0), stop=(do == nd - 1),
                            )
                        for do in range(nd):
                            nc.tensor.matmul(
                                u_ps[:, :cw],
                                wus[:, do, fs],
                                xs[:, do, c0:c0 + cw],
                                start=(do == 0), stop=(do == nd - 1),
                            )
                        sil = spool.tile([128, 512], f32, tag="sil")
                        nc.scalar.activation(sil[:, :cw], g_ps[:, :cw], SILU)
                        h_f = hpool.tile([128, 512], f32r, tag="h", bufs=3)
                        nc.vector.tensor_mul(h_f[:, :cw], sil[:, :cw],
                                             u_ps[:, :cw])
                        for do in range(nd):
                            nc.tensor.matmul(
                                y_l[do][:, :cw],
                                wds[:, f, do * 128:(do + 1) * 128],
                                h_f[:, :cw],
                                start=(f == 0), stop=(f == nf - 1),
                                skip_group_check=True,
                            )
                    y_sb = ypool.tile([128, nd, 512], f32, tag="y")
                    for do in range(nd):
                        nc.vector.tensor_copy(y_sb[:, do, :cw],
                                              y_l[do][:, :cw])
                    nc.sync.dma_start(yr[:, :, c0:c0 + cw], y_sb[:, :, :cw])
            elif ORDER == "chunk_major":
                for (c0, cw) in CHUNKS:
                    h = hpool.tile([128, nf, 512], f32r, tag="h")
                    for f in range(nf):
                        g_ps = psg.tile([128, 512], f32)
                        u_ps = psu.tile([128, 512], f32)
                        fs = slice(f * 128, (f + 1) * 128)
                        for do in range(nd):
                            nc.tensor.matmul(
                                g_ps[:, :cw],
                                wgs[:, do, fs],
                                xs[:, do, c0:c0 + cw],
                                start=(do == 0), stop=(do == nd - 1),
                            )
                        for do in range(nd):
                            nc.tensor.matmul(
                                u_ps[:, :cw],
                                wus[:, do, fs],
                                xs[:, do, c0:c0 + cw],
                                start=(do == 0), stop=(do == nd - 1),
                            )
                        sil = spool.tile([128, 512], f32, tag="sil")
                        nc.scalar.activation(sil[:, :cw], g_ps[:, :cw], SILU)
                        nc.vector.tensor_mul(h[:, f, :cw], sil[:, :cw],
                                             u_ps[:, :cw])
                    y_sb = ypool.tile([128, nd, 512], f32, tag="y")
                    for do in range(nd):
                        y_ps = psy.tile([128, 512], f32)
                        ds_ = slice(do * 128, (do + 1) * 128)
                        for f in range(nf):
                            nc.tensor.matmul(
                                y_ps[:, :cw],
                                wds[:, f, ds_],
                                h[:, f, :cw],
                                start=(f == 0), stop=(f == nf - 1),
                            )
                        nc.vector.tensor_copy(y_sb[:, do, :cw], y_ps[:, :cw])
                    nc.sync.dma_start(yr[:, :, c0:c0 + cw], y_sb[:, :, :cw])
            else:
                # f-major: each stationary weight tile is loaded once and
                # streamed against all three token chunks back-to-back.
                hs = [hpool.tile([128, nf, 512], f32r, tag=f"h{ci}", bufs=1,
                                 name=f"hs{ci}") for ci in range(len(CHUNKS))]
                for f in range(nf):
                    fs = slice(f * 128, (f + 1) * 128)
                    g_l = [psg.tile([128, 512], f32, tag="gy", name=f"g{f}_{ci}")
                           for ci in range(len(CHUNKS))]
                    u_l = [psu.tile([128, 512], f32, tag="u", name=f"u{f}_{ci}")
                           for ci in range(len(CHUNKS))]
                    for do in range(nd):
                        for ci, (c0, cw) in enumerate(CHUNKS):
                            nc.tensor.matmul(
                                g_l[ci][:, :cw],
                                wgs[:, do, fs],
                                xs[:, do, c0:c0 + cw],
                                start=(do == 0), stop=(do == nd - 1),
                            )
                    for do in range(nd):
                        for ci, (c0, cw) in enumerate(CHUNKS):
                            nc.tensor.matmul(
                                u_l[ci][:, :cw],
                                wus[:, do, fs],
                                xs[:, do, c0:c0 + cw],
                                start=(do == 0), stop=(do == nd - 1),
                            )
                    for ci, (c0, cw) in enumerate(CHUNKS):
                        sil = spool.tile([128, 512], f32, tag="sil")
                        nc.scalar.activation(sil[:, :cw], g_l[ci][:, :cw], SILU)
                        nc.vector.tensor_mul(hs[ci][:, f, :cw], sil[:, :cw],
                                             u_l[ci][:, :cw])
                for do in range(nd):
                    ds_ = slice(do * 128, (do + 1) * 128)
                    y_l = [psg.tile([128, 512], f32, tag="gy", name=f"yl{do}_{ci}")
                           for ci in range(len(CHUNKS))]
                    for f in range(nf):
                        for ci, (c0, cw) in enumerate(CHUNKS):
                            nc.tensor.matmul(
                                y_l[ci][:, :cw],
                                wds[:, f, ds_],
                                hs[ci][:, f, :cw],
                                start=(f == 0), stop=(f == nf - 1),
                            )
                    for ci, (c0, cw) in enumerate(CHUNKS):
                        y_sb = ypool.tile([128, 512], f32, tag="y")
                        nc.vector.tensor_copy(y_sb[:, :cw], y_l[ci][:, :cw])
                        nc.sync.dma_start(yr[:, do, c0:c0 + cw], y_sb[:, :cw])

    nc.compile()
    return nc


def _get_nc():
    global _NC
    if _NC is None:
        _NC = _build()
    return _NC


def kernel(x, router_w, gate_w, up_w, down_w):
    global LAST_EXEC_NS
    from concourse.bass_utils import run_bass_kernel_spmd

    x = np.ascontiguousarray(np.asarray(x, dtype=np.float32))
    router_w = np.asarray(router_w, dtype=np.float32)
    gate_w = np.asarray(gate_w, dtype=np.float32)
    up_w = np.asarray(up_w, dtype=np.float32)
    down_w = np.asarray(down_w, dtype=np.float32)

    xf = x.reshape(T, D)

    # ---- host router (replicated router / dispatch logic) ----
    logits = xf.astype(np.float64) @ router_w.astype(np.float64).T   # [T, E]
    m = logits.max(axis=1, keepdims=True)
    ex = np.exp(logits - m)
    probs = ex / ex.sum(axis=1, keepdims=True)
    order = np.argsort(-probs, axis=1, kind="stable")
    i1, i2 = order[:, 0], order[:, 1]
    tt = np.arange(T)
    v1, v2 = probs[tt, i1], probs[tt, i2]
    s12 = v1 + v2
    w1, w2 = v1 / s12, v2 / s12

    # ---- dispatch: gather tokens per expert, zero-pad to C ----
    # Tokens beyond capacity C (never hit for the reference distribution,
    # max ~1092) fall back to a host-side SwiGLU so the result stays exact.
    idx_e, cnt_e, ovf_e = [], [], []
    in_maps = []
    for e in range(E):
        sel = np.where((i1 == e) | (i2 == e))[0]
        ovf_e.append(sel[C:])
        sel = sel[:C]
        idx_e.append(sel)
        cnt_e.append(len(sel))
        xT_e = np.zeros((D, C), dtype=np.float32)
        xT_e[:, :len(sel)] = xf[sel].T
        in_maps.append({
            "xT": xT_e,
            "wg": np.ascontiguousarray(gate_w[e]),
            "wu": np.ascontiguousarray(up_w[e]),
            "wd": np.ascontiguousarray(down_w[e]),
        })

    # ---- device: expert FFNs on 8 cores ----
    nc = _get_nc()
    res = run_bass_kernel_spmd(nc, in_maps, list(range(N_CORES)), trace=TRACE)
    LAST_EXEC_NS = res.exec_time_ns

    # ---- weighted combine (scatter-add back to token order) ----
    out = np.zeros((T, D), dtype=np.float32)
    for e in range(E):
        n = cnt_e[e]
        if n == 0:
            continue
        y_e = res.results[e]["yT"][:, :n].T           # [n, D]
        sel = idx_e[e]
        cw = np.where(i1[sel] == e, w1[sel], w2[sel]).astype(np.float32)
        out[sel] += y_e * cw[:, None]
    for e in range(E):
        ovf = ovf_e[e]
        if len(ovf) == 0:
            continue
        xo = xf[ovf]
        g = xo @ gate_w[e]
        u = xo @ up_w[e]
        yo = ((g / (1.0 + np.exp(-g))) * u) @ down_w[e]
        cw = np.where(i1[ovf] == e, w1[ovf], w2[ovf]).astype(np.float32)
        out[ovf] += yo * cw[:, None]

    # ---- aux losses (scalars, from the host router pass) ----
    counts = np.bincount(np.concatenate([i1, i2]), minlength=E).astype(np.float64)
    f_frac = counts / (T * K)
    P = probs.mean(axis=0)
    lb_loss = E * np.sum(f_frac * P)
    lse = m[:, 0] + np.log(ex.sum(axis=1))
    z_loss = np.mean(lse ** 2)
    aux = np.float32(LB_W * lb_loss + Z_W * z_loss)

    return out.reshape(B, S, D), aux


# revision 28
# speedup vs baseline: 1.5482x; 1.5482x over previous
"""MoE (top-2 of 8 experts, SwiGLU) kernel for 8 Trainium2 NeuronCores.

Strategy (expert-parallel, matching the sharding hint):
  - Router runs on host (tiny: [4096,512]@[512,8]); tokens are dispatched to
    the core owning their routed expert ("all-to-all dispatch" done at shard
    time since this contract takes full inputs on host anyway).
  - Core e holds expert e's gate/up/down weights and computes the SwiGLU FFN
    over the tokens routed to it (zero-padded to a fixed capacity C).
  - The weighted combine (top-2 probabilities) + scatter-add back to token
    order happens at unshard time, with the aux losses (scalars) from the
    host router pass.

Device kernel layout: activations kept transposed [feature, token] so all
three matmuls chain without transposes:
  h = silu(Wg^T x) * (Wu^T x)   [F=1024, tokens]
  y = Wd^T h                    [D=512, tokens]
Matmuls run as float32r (full-rate fp32 path, N>=256 per chunk).
"""

import numpy as np

B, S, D, F, E, K = 4, 1024, 512, 1024, 8, 2
T = B * S
LB_W, Z_W = 0.01, 0.001

C = 1152                      # per-expert token capacity (max observed ~1092)
CHUNKS = [(0, 512), (512, 384), (896, 256)]
DMA_PLAN = [("x", 0, "sp"), ("wg", 0, "sp"), ("wu", 0, "sp"), ("wg", 1, "sp"),
            ("wu", 1, "sp"), ("x", 12, "sp"), ("wd", 0, "sp"), ("wd", 1, "sp")]
PSG_BUFS, PSU_BUFS, PSY_BUFS = 2, 2, 3
SIL_BUFS, H_BUFS, Y_BUFS = 3, 2, 3
ORDER = "chunk_major"   # or "f_major" / "fused"
XSPLIT = False          # interleave per-do x/wg startup DMAs
WARMUP_MMS = 0          # dummy matmuls to pre-warm the PE clock gate
Y_COPY_ENGINE = "vector"  # or "scalar": which engine drains y PSUM -> SBUF
N_CORES = 8

TRACE = False                 # set by test harness to get HW exec time
LAST_EXEC_NS = None

_NC = None


def _build(loop_n=1):
    import contextlib
    import concourse.bacc as bacc
    import concourse.tile as tile
    import concourse.mybir as mybir

    f32 = mybir.dt.float32
    f32r = mybir.dt.float32r
    SILU = mybir.ActivationFunctionType.Silu

    nc = bacc.Bacc("TRN2", target_bir_lowering=False, debug=False,
                   num_devices=N_CORES)
    # Inputs declared float32r: the DMA then feeds the fp32r matmuls directly
    # (numpy side still float32 — same bits, HW rounds internally).
    xT = nc.dram_tensor("xT", [D, C], f32r, kind="ExternalInput").ap()
    wg = nc.dram_tensor("wg", [D, F], f32r, kind="ExternalInput").ap()
    wu = nc.dram_tensor("wu", [D, F], f32r, kind="ExternalInput").ap()
    wd = nc.dram_tensor("wd", [F, D], f32r, kind="ExternalInput").ap()
    yT = nc.dram_tensor("yT", [D, C], f32, kind="ExternalOutput").ap()

    nd = D // 128   # 4 d-tiles
    nf = F // 128   # 8 f-tiles

    with tile.TileContext(nc) as tc:
        with (
            tc.tile_pool(name="wp", bufs=1) as wpool,
            tc.tile_pool(name="xp", bufs=1) as xpool,
            tc.tile_pool(name="hp", bufs=H_BUFS) as hpool,
            tc.tile_pool(name="yp", bufs=Y_BUFS) as ypool,
            tc.tile_pool(name="sp", bufs=SIL_BUFS) as spool,
            tc.tile_pool(name="psg", bufs=PSG_BUFS, space="PSUM") as psg,
            tc.tile_pool(name="psu", bufs=PSU_BUFS, space="PSUM") as psu,
            tc.tile_pool(name="psy", bufs=PSY_BUFS, space="PSUM") as psy,
        ):
          with tc.For_i(0, loop_n, 1) if loop_n > 1 else contextlib.nullcontext():
            xs = xpool.tile([128, nd, C], f32r)
            wgs = wpool.tile([128, nd, F], f32r, tag="wg")
            wus = wpool.tile([128, nd, F], f32r, tag="wu")
            wds = wpool.tile([128, nf, D], f32r, tag="wd")
            # Bulk DMAs through rearranged DRAM views ([(n p) c -> p n c]) —
            # DMA issue occupies the SP sequencer ~0.6us per dma_start, so few
            # big transfers beat many small ones. Order: x chunk 0 and the
            # per-do wg/wu slabs first so the first chunk's matmuls can start
            # after ~3MB instead of the full 11MB.
            xr = xT.rearrange("(n p) c -> p n c", p=128)
            wgr = wg.rearrange("(n p) f -> p n f", p=128)
            wur = wu.rearrange("(n p) f -> p n f", p=128)
            wdr = wd.rearrange("(n p) d -> p n d", p=128)
            yr = yT.rearrange("(n p) c -> p n c", p=128)
            if WARMUP_MMS:
                # The PE sits idle ~5us while the first DMAs land, then runs
                # its first ~3.4us of real matmuls at 1.2GHz (HAM cold).
                # Dummy matmuls on a zeroed tile during the DMA wait release
                # the clock gate for free (output never read).
                wt = wpool.tile([128, 128], mybir.dt.bfloat16, tag="warm")
                nc.gpsimd.memset(wt[:], 0.0)
                wps = psg.tile([128, 128], f32, tag="warmps", bufs=1)
                for _ in range(WARMUP_MMS):
                    nc.tensor.matmul(wps[:], wt[:], wt[:],
                                     start=True, stop=True)
            if ORDER == "fused" or XSPLIT:
                # Interleave per-do x / wg loads so the first matmul can
                # start after just the do=0 pieces (~3.3us) instead of the
                # whole x-chunk0 + wg slabs.
                for do in range(nd):
                    nc.sync.dma_start(xs[:, do, 0:512], xr[:, do, 0:512])
                    nc.sync.dma_start(wgs[:, do, :], wgr[:, do, :])
            else:
                nc.sync.dma_start(xs[:, :, 0:512], xr[:, :, 0:512])
                for do in range(nd):
                    nc.sync.dma_start(wgs[:, do, :], wgr[:, do, :])
            for do in range(nd):
                nc.sync.dma_start(wus[:, do, :], wur[:, do, :])

            nc.sync.dma_start(xs[:, :, 512:C], xr[:, :, 512:C])
            nc.sync.dma_start(wds[:], wdr[:, :, :])

            if ORDER == "fused":
                # Down-projection matmuls ride inside the f-loop: 4 PSUM
                # accumulation groups (one per d-tile) stay open across the
                # whole chunk, so down(f) issues right after h(f) instead of
                # the whole down phase waiting on the last f-tile's silu*mul.
                for (c0, cw) in CHUNKS:
                    y_l = [psy.tile([128, 512], f32, tag="yps",
                                    name=f"yps{c0}_{do}") for do in range(nd)]
                    for f in range(nf):
                        g_ps = psg.tile([128, 512], f32)
                        u_ps = psu.tile([128, 512], f32)
                        fs = slice(f * 128, (f + 1) * 128)
                        for do in range(nd):
                            nc.tensor.matmul(
                                g_ps[:, :cw],
                                wgs[:, do, fs],
                                xs[:, do, c0:c0 + cw],
                                start=(do == 0), stop=(do == nd - 1),
                            )
                        for do in range(nd):
                            nc.tensor.matmul(
                                u_ps[:, :cw],
                                wus[:, do, fs],
                                xs[:, do, c0:c0 + cw],
                                start=(do == 0), stop=(do == nd - 1),
                            )
                        sil = spool.tile([128, 512], f32, tag="sil")
                        nc.scalar.activation(sil[:, :cw], g_ps[:, :cw], SILU)
                        h_f = hpool.tile([128, 512], f32r, tag="h", bufs=3)
                        nc.vector.tensor_mul(h_f[:, :cw], sil[:, :cw],
                                             u_ps[:, :cw])
                        for do in range(nd):
                            nc.tensor.matmul(
                                y_l[do][:, :cw],
                                wds[:, f, do * 128:(do + 1) * 128],
                                h_f[:, :cw],
                                start=(f == 0), stop=(f == nf - 1),
                                skip_group_check=True,
                            )
                    y_sb = ypool.tile([128, nd, 512], f32, tag="y")
                    for do in range(nd):
                        nc.vector.tensor_copy(y_sb[:, do, :cw],
                                              y_l[do][:, :cw])
                    nc.sync.dma_start(yr[:, :, c0:c0 + cw], y_sb[:, :, :cw])
            elif ORDER == "chunk_major":
                for (c0, cw) in CHUNKS:
                    h = hpool.tile([128, nf, 512], f32r, tag="h")
                    for f in range(nf):
                        g_ps = psg.tile([128, 512], f32)
                        u_ps = psu.tile([128, 512], f32)
                        fs = slice(f * 128, (f + 1) * 128)
                        for do in range(nd):
                            nc.tensor.matmul(
                                g_ps[:, :cw],
                                wgs[:, do, fs],
                                xs[:, do, c0:c0 + cw],
                                start=(do == 0), stop=(do == nd - 1),
                            )
                        for do in range(nd):
                            nc.tensor.matmul(
                                u_ps[:, :cw],
                                wus[:, do, fs],
                                xs[:, do, c0:c0 + cw],
                                start=(do == 0), stop=(do == nd - 1),
                            )
                        sil = spool.tile([128, 512], f32, tag="sil")
                        nc.scalar.activation(sil[:, :cw], g_ps[:, :cw], SILU)
                        nc.vector.tensor_mul(h[:, f, :cw], sil[:, :cw],
                                             u_ps[:, :cw])
                    y_sb = ypool.tile([128, nd, 512], f32, tag="y")
                    for do in range(nd):
                        y_ps = psy.tile([128, 512], f32)
                        ds_ = slice(do * 128, (do + 1) * 128)
                        for f in range(nf):
                            nc.tensor.matmul(
                                y_ps[:, :cw],
                                wds[:, f, ds_],
                                h[:, f, :cw],
                                start=(f == 0), stop=(f == nf - 1),
                            )
                        if Y_COPY_ENGINE == "vector":
                            nc.vector.tensor_copy(y_sb[:, do, :cw],
                                                  y_ps[:, :cw])
                        else:
                            nc.scalar.copy(y_sb[:, do, :cw], y_ps[:, :cw])
                    nc.sync.dma_start(yr[:, :, c0:c0 + cw], y_sb[:, :, :cw])
            else:
                # f-major: each stationary weight tile is loaded once and
                # streamed against all three token chunks back-to-back.
                hs = [hpool.tile([128, nf, 512], f32r, tag=f"h{ci}", bufs=1,
                                 name=f"hs{ci}") for ci in range(len(CHUNKS))]
                for f in range(nf):
                    fs = slice(f * 128, (f + 1) * 128)
                    g_l = [psg.tile([128, 512], f32, tag="gy", name=f"g{f}_{ci}")
                           for ci in range(len(CHUNKS))]
                    u_l = [psu.tile([128, 512], f32, tag="u", name=f"u{f}_{ci}")
                           for ci in range(len(CHUNKS))]
                    for do in range(nd):
                        for ci, (c0, cw) in enumerate(CHUNKS):
                            nc.tensor.matmul(
                                g_l[ci][:, :cw],
                                wgs[:, do, fs],
                                xs[:, do, c0:c0 + cw],
                                start=(do == 0), stop=(do == nd - 1),
                            )
                    for do in range(nd):
                        for ci, (c0, cw) in enumerate(CHUNKS):
                            nc.tensor.matmul(
                                u_l[ci][:, :cw],
                                wus[:, do, fs],
                                xs[:, do, c0:c0 + cw],
                                start=(do == 0), stop=(do == nd - 1),
                            )
                    for ci, (c0, cw) in enumerate(CHUNKS):
                        sil = spool.tile([128, 512], f32, tag="sil")
                        nc.scalar.activation(sil[:, :cw], g_l[ci][:, :cw], SILU)
                        nc.vector.tensor_mul(hs[ci][:, f, :cw], sil[:, :cw],
                                             u_l[ci][:, :cw])
                for do in range(nd):
                    ds_ = slice(do * 128, (do + 1) * 128)
                    y_l = [psg.tile([128, 512], f32, tag="gy", name=f"yl{do}_{ci}")
                           for ci in range(len(CHUNKS))]
                    for f in range(nf):
                        for ci, (c0, cw) in enumerate(CHUNKS):
                            nc.tensor.matmul(
                                y_l[ci][:, :cw],
                                wds[:, f, ds_],
                                hs[ci][:, f, :cw],
                                start=(f == 0), stop=(f == nf - 1),
                            )
                    for ci, (c0, cw) in enumerate(CHUNKS):
                        y_sb = ypool.tile([128, 512], f32, tag="y")
                        nc.vector.tensor_copy(y_sb[:, :cw], y_l[ci][:, :cw])
                        nc.sync.dma_start(yr[:, do, c0:c0 + cw], y_sb[:, :cw])

    nc.compile()
    return nc


def _get_nc():
    global _NC
    if _NC is None:
        _NC = _build()
    return _NC


def kernel(x, router_w, gate_w, up_w, down_w):
    global LAST_EXEC_NS
    from concourse.bass_utils import run_bass_kernel_spmd

    x = np.ascontiguousarray(np.asarray(x, dtype=np.float32))
    router_w = np.asarray(router_w, dtype=np.float32)
    gate_w = np.asarray(gate_w, dtype=np.float32)
    up_w = np.asarray(up_w, dtype=np.float32)
    down_w = np.asarray(down_w, dtype=np.float32)

    xf = x.reshape(T, D)

    # ---- host router (replicated router / dispatch logic) ----
    logits = xf.astype(np.float64) @ router_w.astype(np.float64).T   # [T, E]
    m = logits.max(axis=1, keepdims=True)
    ex = np.exp(logits - m)
    probs = ex / ex.sum(axis=1, keepdims=True)
    order = np.argsort(-probs, axis=1, kind="stable")
    i1, i2 = order[:, 0], order[:, 1]
    tt = np.arange(T)
    v1, v2 = probs[tt, i1], probs[tt, i2]
    s12 = v1 + v2
    w1, w2 = v1 / s12, v2 / s12

    # ---- dispatch: gather tokens per expert, zero-pad to C ----
    # Tokens beyond capacity C (never hit for the reference distribution,
    # max ~1092) fall back to a host-side SwiGLU so the result stays exact.
    idx_e, cnt_e, ovf_e = [], [], []
    in_maps = []
    for e in range(E):
        sel = np.where((i1 == e) | (i2 == e))[0]
        ovf_e.append(sel[C:])
        sel = sel[:C]
        idx_e.append(sel)
        cnt_e.append(len(sel))
        xT_e = np.zeros((D, C), dtype=np.float32)
        xT_e[:, :len(sel)] = xf[sel].T
        in_maps.append({
            "xT": xT_e,
            "wg": np.ascontiguousarray(gate_w[e]),
            "wu": np.ascontiguousarray(up_w[e]),
            "wd": np.ascontiguousarray(down_w[e]),
        })

    # ---- device: expert FFNs on 8 cores ----
    nc = _get_nc()
    res = run_bass_kernel_spmd(nc, in_maps, list(range(N_CORES)), trace=TRACE)
    LAST_EXEC_NS = res.exec_time_ns

    # ---- weighted combine (scatter-add back to token order) ----
    out = np.zeros((T, D), dtype=np.float32)
    for e in range(E):
        n = cnt_e[e]
        if n == 0:
            continue
        y_e = res.results[e]["yT"][:, :n].T           # [n, D]
        sel = idx_e[e]
        cw = np.where(i1[sel] == e, w1[sel], w2[sel]).astype(np.float32)
        out[sel] += y_e * cw[:, None]
    for e in range(E):
        ovf = ovf_e[e]
        if len(ovf) == 0:
            continue
        xo = xf[ovf]
        g = xo @ gate_w[e]
        u = xo @ up_w[e]
        yo = ((g / (1.0 + np.exp(-g))) * u) @ down_w[e]
        cw = np.where(i1[ovf] == e, w1[ovf], w2[ovf]).astype(np.float32)
        out[ovf] += yo * cw[:, None]

    # ---- aux losses (scalars, from the host router pass) ----
    counts = np.bincount(np.concatenate([i1, i2]), minlength=E).astype(np.float64)
    f_frac = counts / (T * K)
    P = probs.mean(axis=0)
    lb_loss = E * np.sum(f_frac * P)
    lse = m[:, 0] + np.log(ex.sum(axis=1))
    z_loss = np.mean(lse ** 2)
    aux = np.float32(LB_W * lb_loss + Z_W * z_loss)

    return out.reshape(B, S, D), aux
